# revision 21
# baseline (speedup 1.0000x reference)
"""Block-causal (frame-windowed) attention layer for Trainium2, 8-core SPMD.

Reference computation (B=4, T=2048, C=512, H=8, Dh=64, NPATCH=256):
  LayerNorm(x) -> qkv = xn @ w_qkv -> per-head attention with mask
  frame(i) >= frame(j), frame = idx // 256 -> out @ w_out + b_out

Sharding: core c handles batch c//2 and heads (c%2)*4 .. (c%2)*4+3.
Each core computes a partial y (its heads' contribution to out @ w_out);
the host sums the two partials per batch and adds b_out.

Design (v2):
 - x is shipped twice from host: [T, C] fp16 (for LN stats) and pre-
   transposed [C, T] fp16 (for the QKV matmuls) - no PE transposes.
 - LayerNorm is folded into the QKV matmuls: gamma into the weights
   (host), mean-centering via an extra K=1 accumulation row
   (-colsum(W) x mu^T), rstd via one DVE multiply on the q chunks,
   via the activation scale operand on the k side (per-key partition
   scale on the exp), and via a per-partition tensor_scalar on v.
 - rstd = Newton rsqrt on DVE (no sqrt table load; exp table preloaded
   by a dummy activation at t=0).
 - Attention processes query-frame PAIRS (512 tokens) with N=512
   matmul streams; S matmuls for the head pair (partitions 0-63 /
   64-127) are emitted adjacently so they run concurrently on the PE
   via row tile_position.
 - exp is batched per key chunk across the head pair (FD=1024).
 - S^T layout [keys, queries]; softmax denominator via ones-column in V.
 - Stage B (QKV) / C (attention) / D (out-proj) interleaved per frame
   pair so TensorE, ScalarE, VectorE, GpSimd overlap.
"""

import sys

sys.path.insert(0, "/opt/trn_rl_repo")

import numpy as np

import concourse.bacc as bacc
import concourse.bass as bass
import concourse.mybir as mybir
import concourse.tile as tile
from concourse.bass_utils import run_bass_kernel_spmd

B, T, C = 4, 2048, 512
HEADS, DH = 8, 64
NPATCH = 256
EPS = 1e-5
N_CORES = 8
HPC = HEADS // 2          # heads per core = 4
QK_COLS = HPC * DH * 2    # 512 (q block + k block)
V_COLS = HPC * DH         # 256
NT = T // 128             # 16 token tiles
NCC = C // 128            # 4 contraction chunks
NPAIR = 4                 # query-frame pairs (512 tokens each)

F32 = mybir.dt.float32
FP16 = mybir.dt.float16
AF = mybir.ActivationFunctionType
ALU = mybir.AluOpType

_cache = {}
_run_opts = {}      # test harness may set {"trace": True, ...}
_last_res = [None]  # last BassKernelResults, for profiling


def _build(with_bias: bool, dbg: bool = False):
    nc = bacc.Bacc("TRN2", target_bir_lowering=False, debug=False,
                   num_devices=N_CORES)
    x_d = nc.dram_tensor("x", [T, C], FP16, kind="ExternalInput").ap()
    xt_d = nc.dram_tensor("xt", [C, T], FP16, kind="ExternalInput").ap()
    wqk_d = nc.dram_tensor("wqk", [C, QK_COLS], FP16, kind="ExternalInput").ap()
    wv_d = nc.dram_tensor("wv", [C, V_COLS], FP16, kind="ExternalInput").ap()
    wo_d = nc.dram_tensor("wo", [V_COLS, C], FP16, kind="ExternalInput").ap()
    wqks_d = nc.dram_tensor("wqks", [1, QK_COLS], FP16, kind="ExternalInput").ap()
    wvs_d = nc.dram_tensor("wvs", [1, V_COLS], FP16, kind="ExternalInput").ap()
    if with_bias:
        bqk_d = nc.dram_tensor("bqk", [1, QK_COLS], F32, kind="ExternalInput").ap()
        bv_d = nc.dram_tensor("bv", [1, V_COLS], FP16, kind="ExternalInput").ap()
    y_d = nc.dram_tensor("y", [T, C], F32, kind="ExternalOutput").ap()
    dbg_d = None
    if dbg:
        dbg_d = {
            "d_qkT": nc.dram_tensor("d_qkT", [QK_COLS, T], FP16,
                                    kind="ExternalOutput").ap(),
            "d_muT": nc.dram_tensor("d_muT", [1, T], FP16,
                                    kind="ExternalOutput").ap(),
            "d_rstdT": nc.dram_tensor("d_rstdT", [1, T], FP16,
                                      kind="ExternalOutput").ap(),
            "d_bc": nc.dram_tensor("d_bc", [128, T], FP16,
                                   kind="ExternalOutput").ap(),
            "d_v": nc.dram_tensor("d_v", [128, NT * HPC * (DH + 1)], FP16,
                                  kind="ExternalOutput").ap(),
            "d_oT": nc.dram_tensor("d_oT", [128, 2 * T], FP16,
                                   kind="ExternalOutput").ap(),
            "d_rec": nc.dram_tensor("d_rec", [1, 16 * 512], F32,
                                    kind="ExternalOutput").ap(),
            "d_pt": nc.dram_tensor("d_pt", [128, 2048], FP16,
                                   kind="ExternalOutput").ap(),
            "d_st": nc.dram_tensor("d_st", [128, 2048], F32,
                                   kind="ExternalOutput").ap(),
        }

    with tile.TileContext(nc) as tc:
        _emit(nc, tc, x_d, xt_d, wqk_d, wv_d, wo_d, wqks_d, wvs_d, y_d,
              (bqk_d, bv_d) if with_bias else None, dbg_d)
    nc.compile()
    return nc


def _emit(nc, tc, x_d, xt_d, wqk_d, wv_d, wo_d, wqks_d, wvs_d, y_d, biases,
          dbg_d=None):
    from contextlib import ExitStack
    ctx = ExitStack()
    with ctx:
        singles = ctx.enter_context(tc.tile_pool(name="singles", bufs=1))
        stp = ctx.enter_context(tc.tile_pool(name="stp", bufs=3))
        ptp = ctx.enter_context(tc.tile_pool(name="ptp", bufs=2))
        recips = ctx.enter_context(tc.tile_pool(name="recips", bufs=3))
        yp = ctx.enter_context(tc.tile_pool(name="yp", bufs=3))
        ps_a = ctx.enter_context(tc.tile_pool(name="ps_a", bufs=2, space="PSUM"))
        ps_st = ctx.enter_context(tc.tile_pool(name="ps_st", bufs=2, space="PSUM"))
        ps_pv = ctx.enter_context(tc.tile_pool(name="ps_pv", bufs=2, space="PSUM"))

        # ---- persistent tiles ----
        wqk = singles.tile([128, NCC, QK_COLS], FP16)
        wv = singles.tile([128, NCC, V_COLS], FP16)
        wo = singles.tile([128, 2, C], FP16)
        wqks = singles.tile([1, QK_COLS], FP16)
        wvs = singles.tile([1, V_COLS], FP16)
        nc.sync.dma_start(
            out=wqk, in_=wqk_d.rearrange("(cc p) n -> p cc n", p=128))
        nc.sync.dma_start(
            out=wv, in_=wv_d.rearrange("(cc p) n -> p cc n", p=128))
        nc.sync.dma_start(
            out=wo, in_=wo_d.rearrange("(i p) n -> p i n", p=128))
        nc.sync.dma_start(out=wqks, in_=wqks_d)
        nc.sync.dma_start(out=wvs, in_=wvs_d)
        if biases is not None:
            bqk_d, bv_d = biases
            bqk_sb = singles.tile([128, NCC, 1], F32)
            nc.gpsimd.dma_start(
                out=bqk_sb, in_=bqk_d.rearrange("o (d p) -> p d o", p=128))
            bv_sb = singles.tile([128, V_COLS], FP16)
            nc.gpsimd.dma_start(out=bv_sb, in_=bv_d.to_broadcast((128, V_COLS)))

        eps_t = singles.tile([128, 1], F32)
        nc.vector.memset(eps_t, EPS)
        ones128 = singles.tile([128, 1], FP16)
        nc.vector.memset(ones128, 1.0)
        scr1 = singles.tile([128, 1], F32)
        # dummy exp: pulls the exp table load off the critical path
        nc.scalar.activation(out=scr1, in_=eps_t, func=AF.Exp)

        # big persistent activations
        xst = singles.tile([128, NT, C], FP16)       # x, token-major (stats)
        xT = singles.tile([128, NCC, T], FP16)       # x^T from host
        qkT = singles.tile([128, NCC, T], FP16)      # d0,d1 = q(h01),q(h23); d2,d3 = k
        v_all = singles.tile([128, NT, HPC, DH + 1], FP16)
        oT = singles.tile([128, 2, T], FP16)
        nc.vector.memset(v_all[:, :, :, DH:DH + 1], 1.0)

        if dbg_d is not None:
            dbg_rec = singles.tile([1, 16 * 512], F32)
            dbg_pt = singles.tile([128, 2048], FP16)
            dbg_st = singles.tile([128, 2048], F32)

        # stats tiles
        mvall = singles.tile([128, NT, 2], F32)
        rstd_all = singles.tile([128, NT], F32)
        rstd16 = singles.tile([128, 128], FP16)
        nc.vector.memset(rstd16, 1.0)                # pad cols for xbar transpose
        stagT = singles.tile([128, 128], FP16)
        rstdT = singles.tile([1, T], FP16)
        muT = singles.tile([1, T], FP16)
        rstd_bcast = singles.tile([128, T], FP16)

        # input DMAs, in halves so stats can start early
        for h in range(2):
            nc.sync.dma_start(
                out=xst[:, 8 * h:8 * h + 8, :],
                in_=x_d.rearrange("(t p) c -> p t c", p=128)[:, 8 * h:8 * h + 8, :])
            nc.sync.dma_start(
                out=xT[:, :, 1024 * h:1024 * h + 1024],
                in_=xt_d.rearrange("(cc p) t -> p cc t", p=128)[:, :, 1024 * h:1024 * h + 1024])

        # ---- stage A: LN stats; rstd via Newton rsqrt on DVE ----
        for h in range(2):
            for t in range(8 * h, 8 * h + 8):
                st6 = stp.tile([128, 6], F32, tag="st6")
                nc.vector.bn_stats(out=st6, in_=xst[:, t, :])
                nc.vector.bn_aggr(out=mvall[:, t, :], in_=st6)
            sl = slice(8 * h, 8 * h + 8)
            a_t = stp.tile([128, 8], F32, tag="nta")
            nc.vector.tensor_scalar(out=a_t, in0=mvall[:, sl, 1:2],
                                    scalar1=EPS, scalar2=None, op0=ALU.add)
            s_t = stp.tile([128, 8], F32, tag="nts")
            nc.vector.reciprocal_approx_fast(out=s_t, in_=a_t)
            for it in range(4):
                t2 = stp.tile([128, 8], F32, tag="ntt")
                nc.vector.tensor_tensor(out=t2, in0=s_t, in1=s_t, op=ALU.mult)
                nc.vector.tensor_tensor(out=t2, in0=t2, in1=a_t, op=ALU.mult)
                nc.vector.tensor_scalar(out=t2, in0=t2, scalar1=-0.5,
                                        scalar2=1.5, op0=ALU.mult, op1=ALU.add)
                dst = rstd_all[:, sl] if it == 3 else stp.tile(
                    [128, 8], F32, tag="nts2")
                nc.vector.tensor_tensor(out=dst, in0=s_t, in1=t2, op=ALU.mult)
                s_t = dst
            nc.vector.tensor_copy(out=rstd16[:, sl], in_=rstd_all[:, sl])
            # transpose [128,128]; rows 8h..8h+8 become valid
            nc.sync.dma_start_transpose(out=stagT, in_=rstd16)
            for t in range(8 * h, 8 * h + 8):
                nc.sync.dma_start(
                    out=rstdT[0:1, t * 128:(t + 1) * 128], in_=stagT[t:t + 1, :])
            nc.gpsimd.partition_broadcast(
                rstd_bcast[:, 1024 * h:1024 * h + 1024],
                rstdT[0:1, 1024 * h:1024 * h + 1024])
            # mu^T via matmul over x^T (exact same fp16 data)
            for n in (2 * h, 2 * h + 1):
                mm = ps_a.tile([128, 512], F32, tag="mm")
                for cc in range(NCC):
                    nc.tensor.matmul(
                        mm[0:1, :], ones128, xT[:, cc, n * 512:(n + 1) * 512],
                        start=(cc == 0), stop=(cc == NCC - 1))
                nc.vector.tensor_scalar(
                    out=muT[0:1, n * 512:(n + 1) * 512], in0=mm[0:1, :],
                    scalar1=1.0 / C, scalar2=None, op0=ALU.mult)

        # ---- stages B/C/D interleaved per query-frame pair ----
        def stage_b(n):
            tok = slice(n * 512, (n + 1) * 512)
            for d in range(NCC):
                mm = ps_a.tile([128, 512], F32, tag="mm")
                for cc in range(NCC):
                    nc.tensor.matmul(
                        mm, wqk[:, cc, d * 128:(d + 1) * 128], xT[:, cc, tok],
                        start=(cc == 0), stop=False)
                nc.tensor.matmul(
                    mm, wqks[0:1, d * 128:(d + 1) * 128], muT[0:1, tok],
                    start=False, stop=True)
                if d < 2:   # q: apply rstd now
                    nc.vector.tensor_tensor(
                        out=qkT[:, d, tok], in0=mm, in1=rstd_bcast[:, tok],
                        op=ALU.mult)
                else:       # k: rstd folded into the exp scale later
                    nc.vector.tensor_copy(out=qkT[:, d, tok], in_=mm)
                if biases is not None:
                    nc.vector.tensor_scalar(
                        out=qkT[:, d, tok], in0=qkT[:, d, tok],
                        scalar1=bqk_sb[:, d, :], scalar2=None, op0=ALU.add)
            for t in range(4 * n, 4 * n + 4):
                mm = ps_a.tile([128, 512], F32, tag="mm")
                for cc in range(NCC):
                    nc.tensor.matmul(
                        mm[:, 0:V_COLS], xT[:, cc, t * 128:(t + 1) * 128],
                        wv[:, cc, :], start=(cc == 0), stop=False)
                nc.tensor.matmul(
                    mm[:, 0:V_COLS], muT[0:1, t * 128:(t + 1) * 128], wvs,
                    start=False, stop=True)
                vdst = v_all[:, t, :, 0:DH]
                nc.vector.tensor_scalar(
                    out=vdst,
                    in0=mm[:, 0:V_COLS].rearrange("p (h d) -> p h d", h=HPC),
                    scalar1=rstd_all[:, t:t + 1], scalar2=None, op0=ALU.mult)
                if biases is not None:
                    nc.vector.tensor_tensor(
                        out=vdst, in0=vdst,
                        in1=bv_sb.rearrange("p (h d) -> p h d", h=HPC),
                        op=ALU.add)

        def stage_c(g):
            qs = g * 512
            nkc = 4 * g + 4          # key chunks incl. 2 masked-frame chunks
            for s in range(2):       # head subrounds: (0,1) then (2,3)
                dq, dk = s, 2 + s
                pv0 = ps_pv.tile([DH + 1, 512], F32, tag="pv")
                pv1 = ps_pv.tile([DH + 1, 512], F32, tag="pv")
                pvs = (pv0, pv1)
                for c in range(nkc):
                    ksl = slice(c * 128, (c + 1) * 128)
                    masked = c >= nkc - 2
                    qsl = slice(qs + 256, qs + 512) if masked else slice(qs, qs + 512)
                    osl = slice(256, 512) if masked else slice(0, 512)
                    st = ps_st.tile([128, 2, 512], F32, tag="st")
                    for r in range(2):       # head-pair rows 0-63 / 64-127
                        po = 64 * r
                        nc.tensor.matmul(
                            st[:, r, osl],
                            qkT[po:po + 64, dk, ksl],
                            qkT[po:po + 64, dq, qsl],
                            start=True, stop=True)
                    pt = ptp.tile([128, 2, 512], FP16, tag="pt")
                    nc.scalar.activation(
                        out=pt[:, :, osl], in_=st[:, :, osl], func=AF.Exp,
                        scale=rstd_all[:, c:c + 1])
                    if dbg_d is not None and g == 0 and s == 0 and c < 2:
                        nc.vector.tensor_copy(
                            out=dbg_pt[:, c * 1024:(c + 1) * 1024],
                            in_=pt.rearrange("p a b -> p (a b)"))
                        nc.vector.tensor_copy(
                            out=dbg_st[:, c * 1024:(c + 1) * 1024],
                            in_=st.rearrange("p a b -> p (a b)"))
                    for r in range(2):
                        nc.tensor.matmul(
                            pvs[r][:, osl],
                            v_all[:, c, 2 * s + r, :],
                            pt[:, r, osl],
                            start=(c == 0), stop=(c == nkc - 1),
                            skip_group_check=True)
                for r in range(2):
                    po = 64 * r
                    ssum = recips.tile([1, 512], F32, tag="ssum")
                    nc.vector.tensor_copy(out=ssum, in_=pvs[r][DH:DH + 1, :])
                    rec = recips.tile([1, 512], F32, tag="rec")
                    nc.vector.reciprocal_approx_fast(out=rec, in_=ssum)
                    if dbg_d is not None:
                        idx = g * 4 + s * 2 + r
                        nc.vector.tensor_copy(
                            out=dbg_rec[0:1, idx * 512:(idx + 1) * 512],
                            in_=rec)
                    rrep = recips.tile([64, 512], F32, tag="rrep")
                    nc.gpsimd.partition_broadcast(rrep, rec)
                    nc.vector.tensor_tensor(
                        out=oT[po:po + 64, s, qs:qs + 512],
                        in0=pvs[r][0:DH, :], in1=rrep, op=ALU.mult)

        def stage_d(g):
            for t in range(4 * g, 4 * g + 4):
                ym = ps_a.tile([128, 512], F32, tag="mm")
                for i in range(2):
                    nc.tensor.matmul(
                        ym, oT[:, i, t * 128:(t + 1) * 128], wo[:, i, :],
                        start=(i == 0), stop=(i == 1))
                ysb = yp.tile([128, 512], F32, tag="ysb")
                nc.vector.tensor_copy(out=ysb, in_=ym)
                nc.gpsimd.dma_start(
                    out=y_d[t * 128:(t + 1) * 128, :], in_=ysb)

        for g in range(NPAIR):
            stage_b(g)
            if g >= 1:
                stage_d(g - 1)
            stage_c(g)
        stage_d(NPAIR - 1)

        if dbg_d is not None:
            nc.sync.dma_start(
                out=dbg_d["d_qkT"].rearrange("(d p) t -> p d t", p=128),
                in_=qkT)
            nc.sync.dma_start(out=dbg_d["d_muT"], in_=muT)
            nc.sync.dma_start(out=dbg_d["d_rstdT"], in_=rstdT)
            nc.sync.dma_start(out=dbg_d["d_bc"], in_=rstd_bcast)
            nc.sync.dma_start(
                out=dbg_d["d_v"],
                in_=v_all.rearrange("p a b c -> p (a b c)"))
            nc.sync.dma_start(
                out=dbg_d["d_oT"], in_=oT.rearrange("p i t -> p (i t)"))
            nc.sync.dma_start(out=dbg_d["d_rec"], in_=dbg_rec)
            nc.sync.dma_start(out=dbg_d["d_pt"], in_=dbg_pt)
            nc.sync.dma_start(out=dbg_d["d_st"], in_=dbg_st)


def kernel(x, ln_gamma, ln_beta, w_qkv, w_out, b_out, mask):
    x = np.asarray(x, dtype=np.float32)
    ln_gamma = np.asarray(ln_gamma, dtype=np.float32)
    ln_beta = np.asarray(ln_beta, dtype=np.float32)
    w_qkv = np.asarray(w_qkv, dtype=np.float32)
    w_out = np.asarray(w_out, dtype=np.float32)
    b_out = np.asarray(b_out, dtype=np.float32)

    inner = HEADS * DH
    wq_all = w_qkv[:, 0:inner] * ln_gamma[:, None]
    wk_all = w_qkv[:, inner:2 * inner] * ln_gamma[:, None]
    wv_all = w_qkv[:, 2 * inner:3 * inner] * ln_gamma[:, None]
    scale = DH ** -0.5
    # beta contribution to q/k/v (exact: qkv = ln(x)@(gamma*W) + beta@W)
    bq_all = ln_beta @ w_qkv[:, 0:inner]
    bk_all = ln_beta @ w_qkv[:, inner:2 * inner]
    bv_all = ln_beta @ w_qkv[:, 2 * inner:3 * inner]
    with_bias = bool(
        np.abs(bq_all).max() > 0 or np.abs(bk_all).max() > 0
        or np.abs(bv_all).max() > 0)

    key = ("prog2", with_bias)
    if key not in _cache:
        import os
        _cache[key] = _build(with_bias,
                             dbg=bool(os.environ.get("KERNEL_DEBUG")))
    nc = _cache[key]

    in_maps = []
    for c in range(N_CORES):
        b = c // 2
        h0 = (c % 2) * HPC
        cols = slice(h0 * DH, (h0 + HPC) * DH)
        wqk_c = np.concatenate([wq_all[:, cols] * scale, wk_all[:, cols]],
                               axis=1).astype(np.float32)
        wv_c = wv_all[:, cols].astype(np.float32)
        x16 = x[b].astype(np.float16)
        m = {
            "x": np.ascontiguousarray(x16),
            "xt": np.ascontiguousarray(x16.T),
            "wqk": np.ascontiguousarray(wqk_c.astype(np.float16)),
            "wv": np.ascontiguousarray(wv_c.astype(np.float16)),
            "wo": np.ascontiguousarray(w_out[cols, :].astype(np.float16)),
            "wqks": np.ascontiguousarray(
                (-wqk_c.sum(axis=0))[None, :].astype(np.float16)),
            "wvs": np.ascontiguousarray(
                (-wv_c.sum(axis=0))[None, :].astype(np.float16)),
        }
        if with_bias:
            bqk_c = np.concatenate([bq_all[cols] * scale, bk_all[cols]])
            m["bqk"] = np.ascontiguousarray(bqk_c[None, :].astype(np.float32))
            m["bv"] = np.ascontiguousarray(
                bv_all[cols][None, :].astype(np.float16))
        in_maps.append(m)

    res = run_bass_kernel_spmd(nc, in_maps, core_ids=list(range(N_CORES)),
                               **_run_opts)
    _last_res[0] = res
    y = np.empty((B, T, C), dtype=np.float32)
    for b in range(B):
        y[b] = res.results[2 * b]["y"] + res.results[2 * b + 1]["y"] + b_out
    return y
